# revision 12
# baseline (speedup 1.0000x reference)
"""Trainium2 Bass kernel for LocalDynamicGraph edge-feature construction.

Per batch element b (one NeuronCore each, data-parallel over B=8):
    out[b, n, c, k] = x[b, idx[b,n,k], c] - x[b, n, c]   for c < 64
    out[b, n, c, k] = x[b, n, c - 64]                    for c >= 64

Strategy (per core):
  - SWDGE dma_gather pulls neighbor rows (256B each) straight from HBM into
    SBUF, one point per partition (dst[i%128, i//128, :] placement with a
    host-precomputed index order). The gather ucode caps num_idxs at 1024,
    so each 128-point block takes two 1024-index calls.
  - The full x (2MB) is staged in SBUF once; center rows are read from it.
  - DVE computes (neighbor - center) writing the (c, k)-interleaved first
    half of the output tile; ACT broadcast-copies the center into the
    second half.
  - HWDGE writes each finished (128 points, 2048 ch*k) tile back as one
    fully contiguous 1MB DMA.
"""

import numpy as np

import concourse.bacc as bacc
import concourse.mybir as mybir
from concourse.tile import TileContext
from concourse.bass_utils import run_bass_kernel_spmd

# Problem constants (hardcoded per contest contract).
B = 8
N = 8192
C = 64
K = 16
P = 128              # partitions / points per output tile
NBLK = N // P        # 64 point-blocks per core
GCALL = 1024         # dma_gather ucode limit: max 1024 indices per call
GPB = (P * K) // GCALL   # gather calls per block (2)
GROWS = GCALL // P       # neighbor rows delivered per call per point (8)
S = GCALL // 16          # wrapped index columns per call (64)

_NC_CACHE = {}


def build_nc():
    # 4 SWDGE queues: dma_gather descriptor generation runs on a per-queue
    # GPSIMD core, so alternating queue_num across calls gives ~3x faster
    # aggregate desc-gen (measured 10.3 -> 3.2 ns/index).
    nc = bacc.Bacc(
        "TRN2",
        target_bir_lowering=False,
        dynamic_dma_scratch_size=32768,
        num_swdge_queues=4,
    )
    x = nc.dram_tensor("x", [N, C], mybir.dt.float32, kind="ExternalInput").ap()
    idxw = nc.dram_tensor(
        "idxw", [P, NBLK * GPB * S], mybir.dt.int16, kind="ExternalInput"
    ).ap()
    out = nc.dram_tensor(
        "out", [N, 2 * C * K], mybir.dt.float32, kind="ExternalOutput"
    ).ap()
    out_blocks = out.rearrange("(nb p) f -> nb p f", p=P)

    with TileContext(nc) as tc:
        with (
            tc.tile_pool(name="const", bufs=1) as const_pool,
            tc.tile_pool(name="gat", bufs=8) as gpool,
            tc.tile_pool(name="outp", bufs=6) as opool,
        ):
            # Load indices in chunks so early gathers aren't gated on the
            # full 2MB index transfer.
            IDX_CHUNKS = 16
            idx_sb = const_pool.tile([P, NBLK * GPB * S], mybir.dt.int16)
            ccols = NBLK * GPB * S // IDX_CHUNKS
            for ch in range(IDX_CHUNKS):
                nc.sync.dma_start(
                    idx_sb[:, ch * ccols : (ch + 1) * ccols],
                    idxw[:, ch * ccols : (ch + 1) * ccols],
                )
            # Whole x staged in SBUF: partition p, free (nb, c) = x[nb*128+p, c]
            xall = const_pool.tile([P, NBLK * C], mybir.dt.float32)
            nc.sync.dma_start(
                xall[:].rearrange("p (nb c) -> p nb c", c=C),
                x.rearrange("(nb p) c -> p nb c", p=P),
            )
            nidx_reg = nc.gpsimd.to_reg(GCALL)
            for nb in range(NBLK):
                gt = gpool.tile([P, K * C], mybir.dt.float32)
                for q in range(GPB):
                    col0 = (nb * GPB + q) * S
                    nc.gpsimd.dma_gather(
                        out_ap=gt[:, q * GROWS * C : (q + 1) * GROWS * C].rearrange(
                            "p (g c) -> p g c", c=C
                        ),
                        in_ap=x,
                        idxs_ap=idx_sb[:, col0 : col0 + S],
                        num_idxs=GCALL,
                        num_idxs_reg=nidx_reg,
                        elem_size=C,
                        queue_num=(nb * GPB + q) % 4,
                        single_packet=False,
                    )
                ot = opool.tile([P, 2 * C * K], mybir.dt.float32)
                neigh = (
                    gt[:].rearrange("p (r c) -> p r c", c=C).transpose([0, 2, 1])
                )  # (P, C, K) strided view of the k-major gathered rows
                centr = xall[:, nb * C : (nb + 1) * C]  # (P, C)
                centr_b = centr.unsqueeze(2).broadcast_to([P, C, K])
                dst1 = ot[:, 0 : C * K].rearrange("p (c k) -> p c k", k=K)
                dst2 = ot[:, C * K : 2 * C * K].rearrange("p (c k) -> p c k", k=K)
                nc.vector.tensor_sub(dst1, neigh, centr_b)
                nc.scalar.copy(dst2, centr_b)
                nc.sync.dma_start(out_blocks[nb], ot[:])
    nc.compile()
    return nc


def get_nc():
    if "nc" not in _NC_CACHE:
        _NC_CACHE["nc"] = build_nc()
    return _NC_CACHE["nc"]


def _prep_indices(idx: np.ndarray) -> np.ndarray:
    """int (B, N, K) neighbor indices -> wrapped int16 (B, 128, NBLK*GPB*S)
    SWDGE gather index tensors (per core).

    Gather call (nb, q) covers neighbor rows g in [q*GROWS, (q+1)*GROWS) of
    point block nb. Logical index j of that call (j = g_local*128 + p) must
    hold idx[nb*128 + p, q*GROWS + g_local], so gathered row j lands in
    partition j%128 == p at free slot j//128 == g_local. SWDGE reads index
    j from partition j%16, column j//16 (replicated across all eight
    16-partition GPSIMD core groups).
    """
    idx16 = idx.astype(np.int16)  # (B, N, K)
    arr = idx16.reshape(B, NBLK, P, GPB, GROWS)
    seq = arr.transpose(0, 1, 3, 4, 2)  # (B, nb, q, g, p)
    seq = seq.reshape(B, NBLK, GPB, GCALL)
    wrapped = seq.reshape(B, NBLK, GPB, S, 16).transpose(0, 1, 2, 4, 3)
    # replicate across the eight 16-partition groups -> (B, nb, q, 128, S)
    rep = np.broadcast_to(
        wrapped[:, :, :, None, :, :], (B, NBLK, GPB, 8, 16, S)
    ).reshape(B, NBLK, GPB, P, S)
    idxw = rep.transpose(0, 3, 1, 2, 4).reshape(B, P, NBLK * GPB * S)
    return np.ascontiguousarray(idxw)


def run_on_hw(x: np.ndarray, idx: np.ndarray, **spmd_kwargs):
    """Run the bass kernel on 8 NeuronCores. Returns (out, BassKernelResults)."""
    x = np.ascontiguousarray(np.asarray(x, dtype=np.float32))
    idx = np.asarray(idx)
    idxw = _prep_indices(idx)
    in_maps = [{"x": x[b], "idxw": idxw[b]} for b in range(B)]
    res = run_bass_kernel_spmd(get_nc(), in_maps, core_ids=list(range(B)), **spmd_kwargs)
    out = np.stack([r["out"].reshape(N, 2 * C, K) for r in res.results])
    return out, res


def kernel(x: np.ndarray, idx: np.ndarray) -> np.ndarray:
    out, _ = run_on_hw(x, idx)
    return out


# revision 15
# speedup vs baseline: 1.2220x; 1.2220x over previous
"""Trainium2 Bass kernel for LocalDynamicGraph edge-feature construction.

Per batch element b (one NeuronCore each, data-parallel over B=8):
    out[b, n, c, k] = x[b, idx[b,n,k], c] - x[b, n, c]   for c < 64
    out[b, n, c, k] = x[b, n, c - 64]                    for c >= 64

Strategy (per core):
  - SWDGE dma_gather pulls neighbor rows (256B each) straight from HBM into
    SBUF, one point per partition (dst[i%128, i//128, :] placement with a
    host-precomputed index order). The gather ucode caps num_idxs at 1024,
    so each 128-point block takes two 1024-index calls.
  - The full x (2MB) is staged in SBUF once; center rows are read from it.
  - DVE computes (neighbor - center) writing the (c, k)-interleaved first
    half of the output tile; ACT broadcast-copies the center into the
    second half.
  - HWDGE writes each finished (128 points, 2048 ch*k) tile back as one
    fully contiguous 1MB DMA.
"""

import numpy as np

import concourse.bacc as bacc
import concourse.mybir as mybir
from concourse.tile import TileContext
from concourse.bass_utils import run_bass_kernel_spmd

# Problem constants (hardcoded per contest contract).
B = 8
N = 8192
C = 64
K = 16
P = 128              # partitions / points per output tile
NBLK = N // P        # 64 point-blocks per core
GCALL = 1024         # dma_gather ucode limit: max 1024 indices per call
GPB = (P * K) // GCALL   # gather calls per block (2)
GROWS = GCALL // P       # neighbor rows delivered per call per point (8)
S = GCALL // 16          # wrapped index columns per call (64)

_NC_CACHE = {}


def build_nc():
    # 4 SWDGE queues: dma_gather descriptor generation runs on a per-queue
    # GPSIMD core, so alternating queue_num across calls gives ~3x faster
    # aggregate desc-gen (measured 10.3 -> 3.2 ns/index).
    nc = bacc.Bacc(
        "TRN2",
        target_bir_lowering=False,
        dynamic_dma_scratch_size=32768,
        num_swdge_queues=4,
    )
    x = nc.dram_tensor("x", [N, C], mybir.dt.float32, kind="ExternalInput").ap()
    idxw = nc.dram_tensor(
        "idxw", [P, NBLK * GPB * S], mybir.dt.int16, kind="ExternalInput"
    ).ap()
    out = nc.dram_tensor(
        "out", [N, 2 * C * K], mybir.dt.float32, kind="ExternalOutput"
    ).ap()
    out_blocks = out.rearrange("(nb p) f -> nb p f", p=P)

    with TileContext(nc) as tc:
        with (
            tc.tile_pool(name="const", bufs=1) as const_pool,
            tc.tile_pool(name="gat", bufs=8) as gpool,
            tc.tile_pool(name="outp", bufs=6) as opool,
        ):
            # Load indices in chunks so early gathers aren't gated on the
            # full 2MB index transfer.
            IDX_CHUNKS = 16
            idx_sb = const_pool.tile([P, NBLK * GPB * S], mybir.dt.int16)
            ccols = NBLK * GPB * S // IDX_CHUNKS
            for ch in range(IDX_CHUNKS):
                nc.sync.dma_start(
                    idx_sb[:, ch * ccols : (ch + 1) * ccols],
                    idxw[:, ch * ccols : (ch + 1) * ccols],
                )
            # Whole x staged in SBUF: partition p, free (nb, c) = x[nb*128+p, c]
            xall = const_pool.tile([P, NBLK * C], mybir.dt.float32)
            nc.sync.dma_start(
                xall[:].rearrange("p (nb c) -> p nb c", c=C),
                x.rearrange("(nb p) c -> p nb c", p=P),
            )
            nidx_reg = nc.gpsimd.to_reg(GCALL)
            nidx_reg_q = nc.gpsimd.to_reg(GCALL // 2)
            for nb in range(NBLK):
                gt = gpool.tile([P, K * C], mybir.dt.float32)
                if nb == NBLK - 1:
                    # Last block: 4 smaller calls across all 4 queues so the
                    # final gather latency (and thus the kernel tail) shrinks.
                    for qq in range(4):
                        col0 = nb * GPB * S + qq * (S // 2)
                        nc.gpsimd.dma_gather(
                            out_ap=gt[
                                :, qq * (GROWS // 2) * C : (qq + 1) * (GROWS // 2) * C
                            ].rearrange("p (g c) -> p g c", c=C),
                            in_ap=x,
                            idxs_ap=idx_sb[:, col0 : col0 + S // 2],
                            num_idxs=GCALL // 2,
                            num_idxs_reg=nidx_reg_q,
                            elem_size=C,
                            queue_num=qq,
                        )
                else:
                    for q in range(GPB):
                        col0 = (nb * GPB + q) * S
                        nc.gpsimd.dma_gather(
                            out_ap=gt[
                                :, q * GROWS * C : (q + 1) * GROWS * C
                            ].rearrange("p (g c) -> p g c", c=C),
                            in_ap=x,
                            idxs_ap=idx_sb[:, col0 : col0 + S],
                            num_idxs=GCALL,
                            num_idxs_reg=nidx_reg,
                            elem_size=C,
                            queue_num=(nb * GPB + q) % 4,
                        )
                ot = opool.tile([P, 2 * C * K], mybir.dt.float32)
                neigh = (
                    gt[:].rearrange("p (r c) -> p r c", c=C).transpose([0, 2, 1])
                )  # (P, C, K) strided view of the k-major gathered rows
                centr = xall[:, nb * C : (nb + 1) * C]  # (P, C)
                centr_b = centr.unsqueeze(2).broadcast_to([P, C, K])
                dst1 = ot[:, 0 : C * K].rearrange("p (c k) -> p c k", k=K)
                dst2 = ot[:, C * K : 2 * C * K].rearrange("p (c k) -> p c k", k=K)
                nc.vector.tensor_sub(dst1, neigh, centr_b)
                nc.scalar.copy(dst2, centr_b)
                nc.sync.dma_start(out_blocks[nb], ot[:])
    nc.compile()
    return nc


def get_nc():
    if "nc" not in _NC_CACHE:
        _NC_CACHE["nc"] = build_nc()
    return _NC_CACHE["nc"]


def _prep_indices(idx: np.ndarray) -> np.ndarray:
    """int (B, N, K) neighbor indices -> wrapped int16 (B, 128, NBLK*GPB*S)
    SWDGE gather index tensors (per core).

    Gather call (nb, q) covers neighbor rows g in [q*GROWS, (q+1)*GROWS) of
    point block nb. Logical index j of that call (j = g_local*128 + p) must
    hold idx[nb*128 + p, q*GROWS + g_local], so gathered row j lands in
    partition j%128 == p at free slot j//128 == g_local. SWDGE reads index
    j from partition j%16, column j//16 (replicated across all eight
    16-partition GPSIMD core groups).
    """
    idx16 = idx.astype(np.int16)  # (B, N, K)
    arr = idx16.reshape(B, NBLK, P, GPB, GROWS)
    seq = arr.transpose(0, 1, 3, 4, 2)  # (B, nb, q, g, p)
    seq = seq.reshape(B, NBLK, GPB, GCALL)
    wrapped = seq.reshape(B, NBLK, GPB, S, 16).transpose(0, 1, 2, 4, 3)
    # replicate across the eight 16-partition groups -> (B, nb, q, 128, S)
    rep = np.broadcast_to(
        wrapped[:, :, :, None, :, :], (B, NBLK, GPB, 8, 16, S)
    ).reshape(B, NBLK, GPB, P, S)
    idxw = np.ascontiguousarray(
        rep.transpose(0, 3, 1, 2, 4).reshape(B, P, NBLK * GPB * S)
    )
    # Last block uses 4 sub-calls of GCALL//2 indices (GROWS//2 rows each),
    # one per queue; rebuild its column range with that layout.
    S2 = S // 2
    lb = idx16[:, (NBLK - 1) * P :, :]  # (B, 128, K)
    sub = lb.reshape(B, P, 4, GROWS // 2).transpose(0, 2, 3, 1)  # (B, qq, g, p)
    sub = sub.reshape(B, 4, GCALL // 2)
    subw = sub.reshape(B, 4, S2, 16).transpose(0, 1, 3, 2)  # (B, qq, 16, S2)
    rep2 = np.broadcast_to(subw[:, :, None], (B, 4, 8, 16, S2)).reshape(B, 4, P, S2)
    idxw[:, :, (NBLK - 1) * GPB * S :] = rep2.transpose(0, 2, 1, 3).reshape(
        B, P, 4 * S2
    )
    return idxw


def run_on_hw(x: np.ndarray, idx: np.ndarray, **spmd_kwargs):
    """Run the bass kernel on 8 NeuronCores. Returns (out, BassKernelResults)."""
    x = np.ascontiguousarray(np.asarray(x, dtype=np.float32))
    idx = np.asarray(idx)
    idxw = _prep_indices(idx)
    in_maps = [{"x": x[b], "idxw": idxw[b]} for b in range(B)]
    res = run_bass_kernel_spmd(get_nc(), in_maps, core_ids=list(range(B)), **spmd_kwargs)
    out = np.stack([r["out"].reshape(N, 2 * C, K) for r in res.results])
    return out, res


def kernel(x: np.ndarray, idx: np.ndarray) -> np.ndarray:
    out, _ = run_on_hw(x, idx)
    return out
